# revision 9
# baseline (speedup 1.0000x reference)
"""BitLinear (int8-activation x ternary-weight) matmul on 8 TRN2 NeuronCores.

Full inputs: x [4, 4096, 2048] f32, weight [2048, 2048] f32.
Output: [4, 4096, 2048] fp16 = ((qx @ qw.T) / si / sw).astype(f16).

Strategy: data-parallel over the 16384 rows (2048 rows/core). The
mean|W| pass is SHARDED: each core reads only its own 2 W k-tiles
("wown" input slice), partial-reduces them, and AllReduces the 512-byte
[128,1] partial across the 8 cores via the HBM collective; sw is ready
~20us in instead of ~70us. The full W read (16 tiles on the sync DMA
queue) is stream-quantized tile-by-tile as it arrives -- one DVE
magic-round (v*sw + 1.5*2^23 rounds RNE) plus one ACT Sign(u - MAGIC)
emitting fp8 {-1,0,1} directly -- so each arriving k-tile immediately
unlocks 8 matmuls of row tiles 0-1 (PSUM holds at most 2 row tiles'
accumulators). W is never re-read. Activations quantize per-row to int8
held in bf16 (magic-number RNE) with a DMA-xbar block transpose; x
loads + transposes + output stores all ride the scalar DMA queue so the
W stream on the sync queue is never head-of-line blocked. The matmul is
bf16(qx^T) x fp8(qw^T) with fp32 PSUM accumulation (exact), dequant
fused into the PSUM->SBUF fp16 copy on ACT. Junk matmuls keep the PE's
HAM clock gate warm through the ~25us head, with the mean-broadcast
matmul sandwiched mid-junk so it is not head-of-line blocked. Host only
reshapes/shards and transposes W (layout prep, no math).
"""

import numpy as np

import concourse.mybir as mybir
import concourse.tile as tile
from concourse import bacc
from concourse.bass import ts
from concourse.bass_utils import run_bass_kernel_spmd

N_CORES = 8
ROWS_TOTAL = 4 * 4096
K = 2048
N = 2048
MAGIC = 12582912.0  # 1.5*2^23: fp32 round-to-nearest-even (both signs)
NWARM1 = 75  # junk matmuls before the mean-broadcast matmul
NWARM2 = 20  # junk matmuls after it
NPRE = 4  # x tiles prefetched/quantized before the ramp

f32 = mybir.dt.float32
bf16 = mybir.dt.bfloat16
f16 = mybir.dt.float16
fp8 = mybir.dt.float8e4
Alu = mybir.AluOpType
Act = mybir.ActivationFunctionType
AxX = mybir.AxisListType.X


def build(rows_per_core=ROWS_TOTAL // N_CORES):
    nc = bacc.Bacc(
        "TRN2", target_bir_lowering=False, debug=False, num_devices=N_CORES
    )
    x_ext = nc.declare_dram_parameter("x", [rows_per_core, K], f32, isOutput=False)
    wt_ext = nc.declare_dram_parameter("wt", [K, N], f32, isOutput=False)
    # this core's own 2 k-tiles of wt (the pass-1 shard)
    wown_ext = nc.declare_dram_parameter("wown", [256, N], f32, isOutput=False)
    out_ext = nc.declare_dram_parameter(
        "out", [rows_per_core, N], f16, isOutput=True
    )

    KT = K // 128
    MT = rows_per_core // 128
    NQ = N // 512

    with tile.TileContext(nc) as tc:
        with (
            tc.tile_pool(name="xin", bufs=4) as xin,  # [128,K] f32 x loads
            tc.tile_pool(name="wld", bufs=8) as wld,  # [128,K] f32 W stream
            tc.tile_pool(name="scaled", bufs=3) as scaled,  # [128,K] f32 ACT out
            tc.tile_pool(name="qtmp", bufs=3) as qtmp,  # qx bf16
            tc.tile_pool(name="qxt", bufs=4) as qxtp,  # [128,KT,128] bf16 x^T
            tc.tile_pool(name="outp", bufs=3) as outp,  # [128,N] f16 results
            tc.tile_pool(name="singles", bufs=1) as singles,
            tc.tile_pool(name="small", bufs=6) as small,  # [128,1] stats
            tc.tile_pool(name="pacc", bufs=8, space="PSUM") as pacc,
            tc.tile_pool(name="dram", bufs=2, space="DRAM") as dram,
        ):
            ones_mat = singles.tile([128, 128], f32)
            nc.vector.memset(ones_mat, 1.0)
            qwT = singles.tile([128, KT, N], fp8)
            warm_src = singles.tile([128, 512], bf16)
            nc.vector.memset(warm_src, 1.0)
            negmagic = singles.tile([128, 1], f32)
            nc.vector.memset(negmagic, -MAGIC)

            # ---- PE warm-up: the HAM clock gate halves the PE clock after
            # ~3.4us idle; junk matmuls rotating through the pacc slots hold
            # 2.4 GHz through the ~25us head until real matmuls flow.
            for wi in range(NWARM1):
                pwarm = pacc.tile([128, 512], f32, tag="acc", name=f"warm{wi}")
                nc.tensor.matmul(
                    pwarm, lhsT=warm_src[:, :128], rhs=warm_src,
                    start=True, stop=True, skip_group_check=True,
                )

            # ---- sharded mean(|W|): own 2 k-tiles -> [128,1] partial ->
            # HBM AllReduce across the 8 cores (512 B payload)
            wsp = small.tile([128, 2], f32, tag="wsp")
            wown_tiles = []
            for i in range(2):
                wt_t = wld.tile([128, N], f32, tag="wld", name=f"wown{i}")
                nc.sync.dma_start(out=wt_t, in_=wown_ext[ts(i, 128), :])
                wown_tiles.append(wt_t)
            for i in range(2):
                nc.vector.tensor_reduce(
                    out=wsp[:, i : i + 1], in_=wown_tiles[i], axis=AxX,
                    op=Alu.add, apply_absolute_value=True,
                )
            wtp = small.tile([128, 1], f32, tag="small")
            nc.vector.tensor_reduce(out=wtp, in_=wsp, axis=AxX, op=Alu.add)
            cc_in = dram.tile([128, 1], f32)
            cc_out = dram.tile([128, 1], f32)
            nc.sync.dma_start(out=cc_in, in_=wtp)
            nc.gpsimd.collective_compute(
                "AllReduce",
                Alu.add,
                replica_groups=[list(range(N_CORES))],
                ins=[cc_in.opt()],
                outs=[cc_out.opt()],
            )
            wtot = small.tile([128, 1], f32, tag="small")

            # ---- x loads + full W stream start immediately on their queues.
            # The wtot read-back (which waits on the collective) is slotted
            # after wt4 so it never head-of-line blocks the W stream start.
            x_pre = {}
            for mi in range(min(NPRE, MT)):
                x_t = xin.tile([128, K], f32, tag="xin", name=f"xpre{mi}")
                nc.scalar.dma_start(out=x_t, in_=x_ext[ts(mi, 128), :])
                x_pre[mi] = x_t
            wstream = {}
            for kt in range(KT):
                wt_t = wld.tile([128, K], f32, tag="wld", name=f"wld{kt}")
                nc.sync.dma_start(out=wt_t, in_=wt_ext[ts(kt, 128), :])
                wstream[kt] = wt_t
                if kt == 4:
                    nc.sync.dma_start(out=wtot, in_=cc_out)

            def x_quant(mi):
                if mi in x_pre:
                    x_t = x_pre[mi]
                else:
                    x_t = xin.tile([128, K], f32, tag="xin", name=f"x{mi}")
                    nc.scalar.dma_start(out=x_t, in_=x_ext[ts(mi, 128), :])
                amax = small.tile([128, 1], f32, tag="small")
                nc.vector.tensor_reduce(
                    out=amax, in_=x_t, axis=AxX, op=Alu.max,
                    apply_absolute_value=True,
                )
                amc = small.tile([128, 1], f32, tag="amc", name=f"amc{mi}")
                nc.vector.tensor_scalar_max(out=amc, in0=amax, scalar1=1e-5)
                rec = small.tile([128, 1], f32, tag="small")
                nc.vector.reciprocal(out=rec, in_=amc)
                si = small.tile([128, 1], f32, tag="small")
                nc.vector.tensor_scalar_mul(out=si, in0=rec, scalar1=127.0)
                xs = scaled.tile([128, K], f32, tag="scaled")
                nc.scalar.activation(out=xs, in_=x_t, func=Act.Copy, scale=si)
                qx = qtmp.tile([128, K], bf16, tag="qtmp")
                nc.vector.tensor_scalar(
                    out=qx, in0=xs, scalar1=MAGIC, scalar2=-MAGIC,
                    op0=Alu.add, op1=Alu.add,
                )
                qxT = qxtp.tile(
                    [128, KT, 128], bf16, tag="qxt", name=f"qxT{mi}"
                )
                nc.scalar.dma_start_transpose(out=qxT, in_=qx)
                return qxT, amc

            # quantize the first NPRE x tiles up front so the ramp->steady
            # transition never waits on an activation chain
            xq = {}
            for mi in range(min(NPRE, MT)):
                xq[mi] = x_quant(mi)

            # ---- mean-broadcast matmul sandwiched mid-junk: ones_mat.T @
            # wtot replicates the grand total across all 128 partitions
            ptot_b = pacc.tile([128, 1], f32, tag="acc", name="ptot_b")
            nc.tensor.matmul(ptot_b, lhsT=ones_mat, rhs=wtot, start=True, stop=True)
            for wi in range(NWARM2):
                pwarm = pacc.tile([128, 512], f32, tag="acc", name=f"warm2_{wi}")
                nc.tensor.matmul(
                    pwarm, lhsT=warm_src[:, :128], rhs=warm_src,
                    start=True, stop=True, skip_group_check=True,
                )

            # meanc = max(mean|W|, 1e-5); sw = 1/meanc; q = meanc/127
            meanc_b = small.tile([128, 1], f32, tag="s1")
            nc.vector.tensor_scalar(
                out=meanc_b,
                in0=ptot_b,
                scalar1=1.0 / (K * N),
                scalar2=1e-5,
                op0=Alu.mult,
                op1=Alu.max,
            )
            sw_b = singles.tile([128, 1], f32)
            nc.vector.reciprocal(out=sw_b, in_=meanc_b)
            q_b = singles.tile([128, 1], f32)
            nc.vector.tensor_scalar_mul(out=q_b, in0=meanc_b, scalar1=1.0 / 127.0)

            # ---- streamed W quantize: u = w*sw + MAGIC (DVE, RNE to int);
            # ACT emits Sign(u - MAGIC) straight to fp8 -- for integer n,
            # sign(n) == clip(n, -1, 1).
            def w_quant(kt):
                wt_t = wstream[kt]
                nc.vector.tensor_scalar(
                    out=wt_t, in0=wt_t, scalar1=sw_b, scalar2=MAGIC,
                    op0=Alu.mult, op1=Alu.add,
                )
                nc.scalar.activation(
                    out=qwT[:, kt, :], in_=wt_t, func=Act.Sign, bias=negmagic
                )

            # ---- main loop over row tiles
            def mm(acc, qxT, kt, nq):
                nc.tensor.matmul(
                    acc, lhsT=qxT[:, kt, :], rhs=qwT[:, kt, ts(nq, 512)],
                    start=(kt == 0), stop=(kt == KT - 1),
                    skip_group_check=True,
                )

            def finish(mi, accs, amc):
                cs = small.tile([128, 1], f32, tag="small")
                nc.vector.tensor_mul(cs, amc, q_b)  # (amax/127)*meanc
                o_t = outp.tile([128, N], f16, tag="outp", name=f"o{mi}")
                for nq in range(NQ):
                    nc.scalar.activation(
                        out=o_t[:, ts(nq, 512)], in_=accs[nq],
                        func=Act.Copy, scale=cs,
                    )
                nc.scalar.dma_start(out=out_ext[ts(mi, 128), :], in_=o_t)

            if MT >= 2:
                # ramp: row tiles 0,1 interleaved across kt so each arriving
                # quantized W k-tile unlocks 8 matmuls
                qxT0, amc0 = xq[0]
                qxT1, amc1 = xq[1]
                accs0 = [
                    pacc.tile([128, 512], f32, tag="acc", name=f"acc_0_{i}")
                    for i in range(NQ)
                ]
                accs1 = [
                    pacc.tile([128, 512], f32, tag="acc", name=f"acc_1_{i}")
                    for i in range(NQ)
                ]
                for kt in range(KT):
                    w_quant(kt)
                    for nq in range(NQ):
                        mm(accs0[nq], qxT0, kt, nq)
                    for nq in range(NQ):
                        mm(accs1[nq], qxT1, kt, nq)
                finish(0, accs0, amc0)
                finish(1, accs1, amc1)
                start_mi = 2
            else:
                for kt in range(KT):
                    w_quant(kt)
                start_mi = 0

            for mi in range(start_mi, MT):
                qxT, amc = xq[mi] if mi in xq else x_quant(mi)
                accs = [
                    pacc.tile([128, 512], f32, tag="acc", name=f"acc_{mi}_{i}")
                    for i in range(NQ)
                ]
                if mi == MT - 1:
                    # nq-inner: each output chunk completes as soon as its
                    # 16 accumulations are done, so the dequant + store
                    # overlap the remaining matmuls (shorter kernel tail)
                    for nq in range(NQ):
                        for kt in range(KT):
                            mm(accs[nq], qxT, kt, nq)
                else:
                    for kt in range(KT):
                        for nq in range(NQ):
                            mm(accs[nq], qxT, kt, nq)
                finish(mi, accs, amc)

    nc.compile()
    return nc


_NC_CACHE = {}


def _get_nc(rows_per_core):
    if rows_per_core not in _NC_CACHE:
        _NC_CACHE[rows_per_core] = build(rows_per_core)
    return _NC_CACHE[rows_per_core]


def run(x, weight, **spmd_kwargs):
    x = np.ascontiguousarray(np.asarray(x, dtype=np.float32))
    weight = np.asarray(weight, dtype=np.float32)
    b, s, k = x.shape
    rows = b * s
    rpc = rows // N_CORES
    xr = x.reshape(rows, k)
    wt = np.ascontiguousarray(weight.T)
    nc = _get_nc(rpc)
    in_maps = [
        {
            "x": xr[i * rpc : (i + 1) * rpc],
            "wt": wt,
            "wown": np.ascontiguousarray(wt[i * 256 : (i + 1) * 256]),
        }
        for i in range(N_CORES)
    ]
    res = run_bass_kernel_spmd(
        nc, in_maps, core_ids=list(range(N_CORES)), **spmd_kwargs
    )
    out = np.concatenate(
        [res.results[i]["out"] for i in range(N_CORES)], axis=0
    )
    return out.reshape(b, s, N), res


def kernel(x, weight):
    out, _ = run(x, weight)
    return out


# revision 11
# speedup vs baseline: 1.2622x; 1.2622x over previous
"""BitLinear (int8-activation x ternary-weight) matmul on 8 TRN2 NeuronCores.

Full inputs: x [4, 4096, 2048] f32, weight [2048, 2048] f32.
Output: [4, 4096, 2048] fp16 = ((qx @ qw.T) / si / sw).astype(f16).

Strategy: data-parallel over the 16384 rows (2048 rows/core). The weight
is replicated; each core computes mean|W| on-device. The W stream (16
k-tiles on the sync DMA queue, nothing ahead of it but x0/x1 on the
scalar queue) is reduced tile-by-tile on the otherwise-idle GPSIMD
engine as it arrives, so sw is ready ~3us after the last W tile lands.
12 of the 16 raw tiles stay cached in SBUF; only the first 4 are
re-read (4 MiB) right behind the stream and quantized last. Quantize is
one DVE magic-round (w*sw + 1.5*2^23, RNE) plus one ACT Sign(u - MAGIC)
emitting fp8 {-1,0,1}; the ACT cadence (~1.7us/tile) paces row tiles
0-1, which interleave their matmuls across kt right behind the
quantize burst (PSUM can hold exactly 2 row tiles' accumulators).
Activations quantize per-row to int8 held in bf16 (magic-number RNE,
scale applied in-place on the x tile by ACT) with a DMA-xbar block
transpose; x loads + transposes + output stores ride the scalar DMA
queue. The matmul is bf16(qx^T) x fp8(qw^T) with fp32 PSUM
accumulation (exact); dequant (acc * amax/127 * mean|W|) is fused into
the PSUM->SBUF fp16 copy on ACT. Junk matmuls keep the PE's HAM clock
gate warm through the W-read head, with the mean-broadcast matmul
sandwiched near the end so it is not head-of-line blocked. Host only
reshapes/shards and transposes W (layout prep, no math).
"""

import numpy as np

import concourse.mybir as mybir
import concourse.tile as tile
from concourse import bacc
from concourse.bass import ts
from concourse.bass_utils import run_bass_kernel_spmd

N_CORES = 8
ROWS_TOTAL = 4 * 4096
K = 2048
N = 2048
MAGIC = 12582912.0  # 1.5*2^23: fp32 round-to-nearest-even (both signs)
NWARM1 = 180  # junk matmuls before the mean-broadcast matmul
NWARM2 = 8  # junk matmuls after it
NPRE = 3  # x tiles prefetched/quantized before the ramp
NCACHE = 12  # wld pool bufs: W tiles resident when sw becomes known

f32 = mybir.dt.float32
bf16 = mybir.dt.bfloat16
f16 = mybir.dt.float16
fp8 = mybir.dt.float8e4
Alu = mybir.AluOpType
Act = mybir.ActivationFunctionType
AxX = mybir.AxisListType.X


def build(rows_per_core=ROWS_TOTAL // N_CORES):
    nc = bacc.Bacc(
        "TRN2", target_bir_lowering=False, debug=False, num_devices=N_CORES
    )
    x_ext = nc.declare_dram_parameter("x", [rows_per_core, K], f32, isOutput=False)
    wt_ext = nc.declare_dram_parameter("wt", [K, N], f32, isOutput=False)
    out_ext = nc.declare_dram_parameter(
        "out", [rows_per_core, N], f16, isOutput=True
    )

    KT = K // 128
    MT = rows_per_core // 128
    NQ = N // 512
    NRR = KT - NCACHE  # re-read tiles (quantized last)
    # quantize order: resident tiles first, re-read tiles last
    qorder = list(range(NRR, KT)) + list(range(NRR))

    with tile.TileContext(nc) as tc:
        with (
            tc.tile_pool(name="xin", bufs=4) as xin,  # [128,K] f32 x loads
            tc.tile_pool(name="wld", bufs=NCACHE) as wld,  # [128,K] f32 W
            tc.tile_pool(name="qtmp", bufs=2) as qtmp,  # qx bf16
            tc.tile_pool(name="qxt", bufs=4) as qxtp,  # [128,KT,128] bf16 x^T
            tc.tile_pool(name="outp", bufs=2) as outp,  # [128,N] f16 results
            tc.tile_pool(name="scr", bufs=2) as scr,  # [128,K] bf16 |w| scratch
            tc.tile_pool(name="singles", bufs=1) as singles,
            tc.tile_pool(name="small", bufs=6) as small,  # [128,1] stats
            tc.tile_pool(name="pacc", bufs=8, space="PSUM") as pacc,
        ):
            ones_mat = singles.tile([128, 128], f32)
            nc.vector.memset(ones_mat, 1.0)
            qwT = singles.tile([128, KT, N], fp8)
            wsums = singles.tile([128, KT], f32)
            warm_src = singles.tile([128, 512], bf16)
            nc.vector.memset(warm_src, 1.0)
            negmagic = singles.tile([128, 1], f32)
            nc.vector.memset(negmagic, -MAGIC)

            # ---- PE warm-up through the W-read head
            for wi in range(NWARM1):
                pwarm = pacc.tile([128, 512], f32, tag="acc", name=f"warm{wi}")
                nc.tensor.matmul(
                    pwarm, lhsT=warm_src[:, :128], rhs=warm_src,
                    start=True, stop=True, skip_group_check=True,
                )

            # ---- x0..x2 first on the scalar queue
            x_pre = {}
            for mi in range(min(NPRE, MT)):
                x_t = xin.tile([128, K], f32, tag="xin", name=f"xpre{mi}")
                nc.scalar.dma_start(out=x_t, in_=x_ext[ts(mi, 128), :])
                x_pre[mi] = x_t

            def x_quant(mi):
                if mi in x_pre:
                    x_t = x_pre[mi]
                else:
                    x_t = xin.tile([128, K], f32, tag="xin", name=f"x{mi}")
                    nc.scalar.dma_start(out=x_t, in_=x_ext[ts(mi, 128), :])
                amax = small.tile([128, 1], f32, tag="small")
                nc.vector.tensor_reduce(
                    out=amax, in_=x_t, axis=AxX, op=Alu.max,
                    apply_absolute_value=True,
                )
                amc = small.tile([128, 1], f32, tag="amc", name=f"amc{mi}")
                nc.vector.tensor_scalar_max(out=amc, in0=amax, scalar1=1e-5)
                rec = small.tile([128, 1], f32, tag="small")
                nc.vector.reciprocal(out=rec, in_=amc)
                si = small.tile([128, 1], f32, tag="small")
                nc.vector.tensor_scalar_mul(out=si, in0=rec, scalar1=127.0)
                # scale in place: x_t *= si (ACT), then RNE round on DVE
                nc.scalar.activation(out=x_t, in_=x_t, func=Act.Copy, scale=si)
                qx = qtmp.tile([128, K], bf16, tag="qtmp")
                nc.vector.tensor_scalar(
                    out=qx, in0=x_t, scalar1=MAGIC, scalar2=-MAGIC,
                    op0=Alu.add, op1=Alu.add,
                )
                qxT = qxtp.tile(
                    [128, KT, 128], bf16, tag="qxt", name=f"qxT{mi}"
                )
                nc.scalar.dma_start_transpose(out=qxT, in_=qx)
                return qxT, amc

            xq = {}
            for mi in range(min(NPRE, MT)):
                xq[mi] = x_quant(mi)

            # ---- W stream on the sync queue; mean via ACT Abs+accum as
            # tiles land (the Abs output itself is scratch)
            wstream = {}
            for kt in range(KT):
                wt_t = wld.tile([128, K], f32, tag="wld", name=f"wld{kt}")
                nc.sync.dma_start(out=wt_t, in_=wt_ext[ts(kt, 128), :])
                wstream[kt] = wt_t
                aw = scr.tile([128, K], bf16, tag="scr")
                nc.scalar.activation(
                    out=aw, in_=wt_t, func=Act.Abs,
                    accum_out=wsums[:, kt : kt + 1],
                )
            # re-read of the first NRR tiles, right behind the stream
            for kt in range(NRR):
                wt_t = wld.tile([128, K], f32, tag="wld", name=f"wldr{kt}")
                nc.sync.dma_start(out=wt_t, in_=wt_ext[ts(kt, 128), :])
                wstream[kt] = wt_t
            wtot = small.tile([128, 1], f32, tag="small")
            nc.vector.tensor_reduce(out=wtot, in_=wsums, axis=AxX, op=Alu.add)

            # ---- mean broadcast + scalar chain: ones_mat.T @ wtot
            # replicates the grand total across all 128 partitions
            ptot_b = pacc.tile([128, 1], f32, tag="acc", name="ptot_b")
            nc.tensor.matmul(ptot_b, lhsT=ones_mat, rhs=wtot, start=True, stop=True)
            for wi in range(NWARM2):
                pwarm = pacc.tile([128, 512], f32, tag="acc", name=f"warm2_{wi}")
                nc.tensor.matmul(
                    pwarm, lhsT=warm_src[:, :128], rhs=warm_src,
                    start=True, stop=True, skip_group_check=True,
                )
            meanc_b = small.tile([128, 1], f32, tag="s1")
            nc.vector.tensor_scalar(
                out=meanc_b,
                in0=ptot_b,
                scalar1=1.0 / (K * N),
                scalar2=1e-5,
                op0=Alu.mult,
                op1=Alu.max,
            )
            sw_b = singles.tile([128, 1], f32)
            nc.vector.reciprocal(out=sw_b, in_=meanc_b)
            q_b = singles.tile([128, 1], f32)
            nc.vector.tensor_scalar_mul(out=q_b, in0=meanc_b, scalar1=1.0 / 127.0)

            # ---- W quantize: u = w*sw + MAGIC (DVE, RNE to int); ACT emits
            # Sign(u - MAGIC) straight to fp8: for integer n, sign(n) ==
            # clip(n, -1, 1).
            def w_quant(kt):
                wt_t = wstream[kt]
                nc.vector.tensor_scalar(
                    out=wt_t, in0=wt_t, scalar1=sw_b, scalar2=MAGIC,
                    op0=Alu.mult, op1=Alu.add,
                )
                nc.scalar.activation(
                    out=qwT[:, kt, :], in_=wt_t, func=Act.Sign, bias=negmagic
                )

            # ---- main loop over row tiles
            def mm(acc, qxT, kt, nq, start, stop):
                nc.tensor.matmul(
                    acc, lhsT=qxT[:, kt, :], rhs=qwT[:, kt, ts(nq, 512)],
                    start=start, stop=stop,
                    skip_group_check=True,
                )

            def finish(mi, accs, amc):
                cs = small.tile([128, 1], f32, tag="small")
                nc.vector.tensor_mul(cs, amc, q_b)  # (amax/127)*meanc
                o_t = outp.tile([128, N], f16, tag="outp", name=f"o{mi}")
                for nq in range(NQ):
                    nc.scalar.activation(
                        out=o_t[:, ts(nq, 512)], in_=accs[nq],
                        func=Act.Copy, scale=cs,
                    )
                nc.scalar.dma_start(out=out_ext[ts(mi, 128), :], in_=o_t)

            if MT >= 2:
                # ramp: row tiles 0,1 interleaved across the quantize burst
                qxT0, amc0 = xq[0]
                qxT1, amc1 = xq[1]
                accs0 = [
                    pacc.tile([128, 512], f32, tag="acc", name=f"acc_0_{i}")
                    for i in range(NQ)
                ]
                accs1 = [
                    pacc.tile([128, 512], f32, tag="acc", name=f"acc_1_{i}")
                    for i in range(NQ)
                ]
                for qi, kt in enumerate(qorder):
                    w_quant(kt)
                    st, sp = qi == 0, qi == KT - 1
                    for nq in range(NQ):
                        mm(accs0[nq], qxT0, kt, nq, st, sp)
                    for nq in range(NQ):
                        mm(accs1[nq], qxT1, kt, nq, st, sp)
                finish(0, accs0, amc0)
                finish(1, accs1, amc1)
                start_mi = 2
            else:
                for kt in qorder:
                    w_quant(kt)
                start_mi = 0

            for mi in range(start_mi, MT):
                qxT, amc = xq[mi] if mi in xq else x_quant(mi)
                accs = [
                    pacc.tile([128, 512], f32, tag="acc", name=f"acc_{mi}_{i}")
                    for i in range(NQ)
                ]
                if mi == MT - 1:
                    # nq-inner: each output chunk completes as soon as its
                    # 16 accumulations are done, so the dequant + store
                    # overlap the remaining matmuls (shorter kernel tail)
                    for nq in range(NQ):
                        for qi, kt in enumerate(qorder):
                            mm(accs[nq], qxT, kt, nq, qi == 0, qi == KT - 1)
                else:
                    for qi, kt in enumerate(qorder):
                        st, sp = qi == 0, qi == KT - 1
                        for nq in range(NQ):
                            mm(accs[nq], qxT, kt, nq, st, sp)
                finish(mi, accs, amc)

    nc.compile()
    return nc


_NC_CACHE = {}


def _get_nc(rows_per_core):
    if rows_per_core not in _NC_CACHE:
        _NC_CACHE[rows_per_core] = build(rows_per_core)
    return _NC_CACHE[rows_per_core]


def run(x, weight, **spmd_kwargs):
    x = np.ascontiguousarray(np.asarray(x, dtype=np.float32))
    weight = np.asarray(weight, dtype=np.float32)
    b, s, k = x.shape
    rows = b * s
    rpc = rows // N_CORES
    xr = x.reshape(rows, k)
    wt = np.ascontiguousarray(weight.T)
    nc = _get_nc(rpc)
    in_maps = [
        {"x": xr[i * rpc : (i + 1) * rpc], "wt": wt} for i in range(N_CORES)
    ]
    res = run_bass_kernel_spmd(
        nc, in_maps, core_ids=list(range(N_CORES)), **spmd_kwargs
    )
    out = np.concatenate(
        [res.results[i]["out"] for i in range(N_CORES)], axis=0
    )
    return out.reshape(b, s, N), res


def kernel(x, weight):
    out, _ = run(x, weight)
    return out
